# revision 1
# baseline (speedup 1.0000x reference)
"""MoE FFN (capacity-routed, top-2, SwiGLU) on 8 TRN2 NeuronCores.

Expert-parallel: one expert per core.  Router (RMSNorm + gate + top-2) is
token-sharded (512 tokens/core) and all-gathered; dispatch/combine are
realized as indirect DMA gather / scatter-add driven by on-device
position (cumsum) computation that reproduces the reference's
per-(expert, k-slot) capacity stream semantics exactly.  Final combine
reduction across experts is a ReduceScatter; host concatenates slices.
"""

import numpy as np

E, K, D, H = 8, 2, 1024, 4096
B, S = 2, 2048
T = B * S                      # 4096
TPC = T // 8                   # 512 tokens per core
CAP = int(1.5 * T * K / E)     # 1536
RMS_EPS = 1e-6
ROW = 1152                     # padded xn row: 1024 x | p0 | p1 | pad  (2304B % 256 == 0)
NS = 3                         # 1536 slots = 3 slices of 512
WRAP = CAP // 16               # 96 wrapped idx cols


def build_bass():
    import concourse.bass as bass
    import concourse.mybir as mybir
    from concourse import bacc, tile

    f32 = mybir.dt.float32
    bf16 = mybir.dt.bfloat16
    i16 = mybir.dt.int16
    u32 = mybir.dt.uint32
    AF = mybir.ActivationFunctionType
    OP = mybir.AluOpType
    AX = mybir.AxisListType
    ts = bass.ts

    nc = bacc.Bacc("TRN2", target_bir_lowering=False, debug=False, num_devices=8)

    xs = nc.dram_tensor("xs", [TPC, D], f32, kind="ExternalInput").ap()
    gw = nc.dram_tensor("gw", [D, E], f32, kind="ExternalInput").ap()
    w1b = nc.dram_tensor("w1b", [128, 32, 8, 128], bf16, kind="ExternalInput").ap()
    w2b = nc.dram_tensor("w2b", [128, 32, 8, 128], bf16, kind="ExternalInput").ap()
    w3b = nc.dram_tensor("w3b", [8, 128, 4, D], bf16, kind="ExternalInput").ap()
    eid = nc.dram_tensor("eid", [16, 1], f32, kind="ExternalInput").ap()
    ident = nc.dram_tensor("ident", [128, 128], f32, kind="ExternalInput").ap()
    out = nc.dram_tensor("out", [TPC, D], f32, kind="ExternalOutput").ap()

    RG = [list(range(8))]

    with tile.TileContext(nc) as tc:
        with (
            tc.tile_pool(name="dram", bufs=1, space="DRAM") as dp,
            tc.tile_pool(name="const", bufs=1) as cst,
            tc.tile_pool(name="lists", bufs=1) as lp,
            tc.tile_pool(name="eip", bufs=1) as eip,
        ):
            # ---- internal DRAM ----
            xn_loc = dp.tile([TPC + 16, ROW], bf16)
            tk_loc = dp.tile([2, TPC], f32)
            xn_full = dp.tile([(TPC + 16) * 8, ROW], bf16, addr_space="Shared")
            tk_full = dp.tile([8, 2, TPC], f32, addr_space="Shared")
            partial = dp.tile([T, D], f32)
            rs_out = dp.tile([TPC, D], f32)
            sl_dram = dp.tile([2, CAP], i16)
            gl_dram = dp.tile([2, CAP], i16)
            gate_dram = dp.tile([2, CAP], bf16)

            # ---- constants ----
            id_sb = cst.tile([128, 128], f32)
            nc.sync.dma_start(id_sb[:], ident)
            gw_sb = cst.tile([128, 8, E], f32)
            nc.sync.dma_start(gw_sb[:], gw.rearrange("(dc p) e -> p dc e", p=128))
            eid_sb = cst.tile([16, 1], f32)
            nc.sync.dma_start(eid_sb[:], eid)
            eps_col = cst.tile([128, 1], f32)
            nc.vector.memset(eps_col[:], RMS_EPS)

            # ---- zero-fill partial + xn_full zero row ----
            zf = cst.tile([128, D], f32)
            nc.vector.memset(zf[:], 0.0)
            for i in range(T // 128):
                nc.scalar.dma_start(partial[ts(i, 128), :], zf[:])
            zpad = cst.tile([16, ROW], bf16)
            nc.vector.memset(zpad[:], 0.0)
            nc.scalar.dma_start(xn_loc[TPC : TPC + 16, :], zpad[:])

            # ---- long-lived small tiles ----
            # idx lists live in (128, n/16) tiles: HW/sim read the wrapped
            # pattern from partitions 0-15; rows 16-127 are zero filler.
            slw = [lp.tile([128, WRAP], i16, name=f"slw{k}") for k in range(2)]
            glw = [lp.tile([128, WRAP], i16, name=f"glw{k}") for k in range(2)]
            ntile = lp.tile([2, 12], mybir.dt.int32, name="ntile")
            gates = [lp.tile([128, NS * 4], bf16, name=f"gates{k}") for k in range(2)]
            gatesf = [lp.tile([128, NS * 4], f32, name=f"gatesf{k}") for k in range(2)]
            ei = eip.tile([128, 8, CAP], bf16)

            # ================= router (local 512 tokens) =================
            with (
                tc.tile_pool(name="rout", bufs=2) as rp,
                tc.tile_pool(name="routc", bufs=4) as rc,
                tc.tile_pool(name="rpsum", bufs=2, space="PSUM") as rps,
            ):
                for i in range(TPC // 128):
                    xt = rp.tile([128, D], f32, tag="xt")
                    nc.sync.dma_start(xt[:], xs[ts(i, 128), :])
                    sq = rps.tile([128, D], f32, tag="sq")
                    ssum = rc.tile([128, 1], f32, tag="ssum")
                    nc.scalar.activation(sq[:], xt[:], AF.Square, accum_out=ssum[:])
                    s1 = rc.tile([128, 1], f32, tag="s1")
                    nc.scalar.activation(
                        s1[:], ssum[:], AF.Sqrt, bias=eps_col[:], scale=1.0 / D
                    )
                    r1 = rc.tile([128, 1], f32, tag="r1")
                    nc.vector.reciprocal(r1[:], s1[:])
                    xnf = rp.tile([128, D], f32, tag="xnf")
                    nc.scalar.activation(xnf[:], xt[:], AF.Copy, scale=r1[:])
                    xnb = rp.tile([128, D], bf16, tag="xnb")
                    nc.vector.tensor_copy(xnb[:], xnf[:])
                    nc.sync.dma_start(xn_loc[ts(i, 128), 0:D], xnb[:])

                    # transpose x_norm tile, then logits = xnT.T @ gw -> (tok, E)
                    xnT = rp.tile([128, 8, 128], f32, tag="xnT")
                    for dc in range(8):
                        tp = rps.tile([128, 128], f32, tag="tp")
                        nc.tensor.transpose(tp[:], xnf[:, ts(dc, 128)], id_sb[:])
                        nc.scalar.copy(xnT[:, dc, :], tp[:])
                    lps = rps.tile([128, E], f32, tag="lps")
                    for dc in range(8):
                        nc.tensor.matmul(
                            lps[:], xnT[:, dc, :], gw_sb[:, dc, :],
                            start=(dc == 0), stop=(dc == 7),
                        )
                    lg = rp.tile([128, E], f32, tag="lg")
                    nc.vector.tensor_copy(lg[:], lps[:])

                    mx = rp.tile([128, 8], f32, tag="mx")
                    nc.vector.max(mx[:], lg[:])
                    mi = rp.tile([128, 8], u32, tag="mi")
                    nc.vector.max_index(mi[:], mx[:], lg[:])

                    negm1 = rc.tile([128, 1], f32, tag="negm1")
                    nc.vector.tensor_scalar_mul(negm1[:], mx[:, 0:1], -1.0)
                    ex = rp.tile([128, E], f32, tag="ex")
                    nc.scalar.activation(ex[:], lg[:], AF.Exp, bias=negm1[:])
                    zz = rc.tile([128, 1], f32, tag="zz")
                    nc.vector.reduce_sum(zz[:], ex[:], axis=AX.X)
                    t2 = rc.tile([128, 1], f32, tag="t2")
                    nc.scalar.activation(t2[:], mx[:, 1:2], AF.Exp, bias=negm1[:])
                    u0 = rc.tile([128, 1], f32, tag="u0")
                    nc.vector.scalar_tensor_tensor(
                        u0[:], zz[:], 1e-10, t2[:], op0=OP.mult, op1=OP.add
                    )
                    u1 = rc.tile([128, 1], f32, tag="u1")
                    nc.vector.tensor_scalar_add(u1[:], u0[:], 1.0)
                    p1 = rc.tile([128, 1], f32, tag="p1")
                    nc.vector.reciprocal(p1[:], u1[:])
                    p2 = rc.tile([128, 1], f32, tag="p2")
                    nc.vector.tensor_mul(p2[:], t2[:], p1[:])

                    idxf = rp.tile([128, 2], f32, tag="idxf")
                    nc.vector.tensor_copy(idxf[:], mi[:, 0:2])
                    nc.scalar.dma_start(tk_loc[0:1, ts(i, 128)], idxf[:, 0:1])
                    nc.scalar.dma_start(tk_loc[1:2, ts(i, 128)], idxf[:, 1:2])

                    p1b = rc.tile([128, 1], bf16, tag="p1b")
                    nc.vector.tensor_copy(p1b[:], p1[:])
                    p2b = rc.tile([128, 1], bf16, tag="p2b")
                    nc.vector.tensor_copy(p2b[:], p2[:])
                    nc.scalar.dma_start(xn_loc[ts(i, 128), D : D + 1], p1b[:])
                    nc.scalar.dma_start(xn_loc[ts(i, 128), D + 1 : D + 2], p2b[:])

            # ================= all-gathers =================
            nc.gpsimd.collective_compute(
                "AllGather", OP.bypass, RG, ins=[xn_loc.opt()],
                outs=[xn_full.opt()],
            )
            nc.gpsimd.collective_compute(
                "AllGather", OP.bypass, RG, ins=[tk_loc.opt()],
                outs=[tk_full.opt()],
            )

            # ================= positions / slot lists =================
            with tc.tile_pool(name="comp", bufs=1) as cp:
                idxr = cp.tile([16, T], f32)
                for b in range(8):
                    eng = nc.sync if b % 2 == 0 else nc.scalar
                    eng.dma_start(
                        idxr[2 * b : 2 * b + 2, :],
                        tk_full.rearrange("r f t -> f r t"),
                    )
                mask = cp.tile([16, T], f32)
                nc.vector.tensor_scalar(
                    out=mask[:], in0=idxr[:], scalar1=eid_sb[:], scalar2=None,
                    op0=OP.is_equal,
                )
                zer16 = cp.tile([16, T], f32)
                nc.vector.memset(zer16[:], 0.0)
                cum = cp.tile([16, T], f32)
                nc.vector.tensor_tensor_scan(
                    cum[:], mask[:], zer16[:], 0.0, op0=OP.add, op1=OP.add
                )
                # per-(stream, slot-tile) valid counts for scatter descriptors
                cnt = cp.tile([2, 1], f32)
                nc.vector.reduce_sum(cnt[:], mask[0:2, :], axis=AX.X)
                nc.vector.tensor_scalar_min(cnt[:], cnt[:], float(CAP))
                srow = cp.tile([2, 12], f32)
                nc.gpsimd.iota(
                    srow[:], pattern=[[-128, 12]], base=0, channel_multiplier=0,
                    allow_small_or_imprecise_dtypes=True,
                )
                ntf = cp.tile([2, 12], f32)
                nc.vector.tensor_scalar(
                    out=ntf[:], in0=srow[:], scalar1=cnt[:], scalar2=None,
                    op0=OP.add,
                )
                nc.vector.tensor_scalar_min(ntf[:], ntf[:], 128.0)
                nc.vector.tensor_scalar_max(ntf[:], ntf[:], 0.0)
                nc.vector.tensor_copy(ntile[:], ntf[:])

                nc.vector.tensor_tensor(
                    out=cum[:], in0=cum[:], in1=mask[:], op=OP.mult
                )
                pos16 = cp.tile([16, T], i16)
                nc.vector.tensor_scalar(
                    out=pos16[:], in0=cum[:], scalar1=-1.0, scalar2=None,
                    op0=OP.add,
                )
                tok16 = cp.tile([16, T], i16)
                nc.gpsimd.iota(
                    tok16[:], pattern=[[1, T]], base=1, channel_multiplier=0
                )
                sraw = cp.tile([16, 2046], i16)
                nc.gpsimd.local_scatter(
                    sraw[:], tok16[:], pos16[:], channels=16, num_elems=2046,
                    num_idxs=T,
                )
                # second scatter carries the 528-block gather row index:
                # iota value = 1 + b*528 + j for token t = b*512 + j
                tokg = cp.tile([16, T], i16)
                nc.gpsimd.iota(
                    tokg[:], pattern=[[TPC + 16, 8], [1, TPC]], base=1,
                    channel_multiplier=0,
                )
                sraw_g = cp.tile([16, 2046], i16)
                nc.gpsimd.local_scatter(
                    sraw_g[:], tokg[:], pos16[:], channels=16, num_elems=2046,
                    num_idxs=T,
                )
                sl = cp.tile([16, CAP], i16)
                nc.vector.tensor_scalar(
                    out=sl[:], in0=sraw[:, 0:CAP], scalar1=-1, scalar2=None,
                    op0=OP.add,
                )
                em = cp.tile([16, CAP], i16)
                nc.vector.tensor_scalar(
                    out=em[:], in0=sraw_g[:, 0:CAP], scalar1=0, scalar2=None,
                    op0=OP.is_equal,
                )
                gl = cp.tile([16, CAP], i16)
                nc.vector.tensor_scalar(
                    out=gl[:], in0=sraw_g[:, 0:CAP], scalar1=-1, scalar2=None,
                    op0=OP.add,
                )
                nc.vector.scalar_tensor_tensor(
                    gl[:], em[:], TPC + 1, gl[:], op0=OP.mult, op1=OP.add
                )
                nc.sync.dma_start(sl_dram[:, :], sl[0:2, :])
                nc.scalar.dma_start(gl_dram[:, :], gl[0:2, :])
                for k in range(2):
                    for b in range(8):
                        eng = nc.sync if b % 2 == 0 else nc.scalar
                        eng.dma_start(
                            slw[k][16 * b : 16 * (b + 1), :],
                            sl_dram[k, :].rearrange("(f p) -> p f", p=16),
                        )
                        eng.dma_start(
                            glw[k][16 * b : 16 * (b + 1), :],
                            gl_dram[k, :].rearrange("(f p) -> p f", p=16),
                        )

            # ================= token gather =================
            with tc.tile_pool(name="gath", bufs=2) as gp:
                for ns in range(NS):
                    gc = []
                    for k in range(2):
                        g = gp.tile([128, 9, 512], bf16, tag=f"g{k}", name=f"g{k}_{ns}")
                        nc.gpsimd.dma_gather(
                            g[:], xn_full[:, :], glw[k][:, ns * 32 : (ns + 1) * 32],
                            num_idxs=512, num_idxs_reg=512, elem_size=ROW,
                            transpose=True,
                        )
                        gc.append(g)
                    nc.vector.tensor_tensor(
                        out=ei[:, :, ts(ns, 512)], in0=gc[0][:, 0:8, :],
                        in1=gc[1][:, 0:8, :], op=OP.add,
                    )
                    for k in range(2):
                        nc.scalar.dma_start(
                            gate_dram[k, ts(ns, 512)], gc[k][k : k + 1, 8, :]
                        )
                for k in range(2):
                    nc.scalar.dma_start(
                        gates[k][:], gate_dram[k, :].rearrange("(f p) -> p f", p=128)
                    )
                    nc.vector.tensor_copy(gatesf[k][:], gates[k][:])

            # ================= expert FFN + combine =================
            with (
                tc.tile_pool(name="wts12", bufs=2) as wp,
                tc.tile_pool(name="wts3", bufs=6) as wp3,
                tc.tile_pool(name="hidp", bufs=1) as hp,
                tc.tile_pool(name="silp", bufs=2) as sp,
                tc.tile_pool(name="scp", bufs=2) as scp,
                tc.tile_pool(name="ps1", bufs=2, space="PSUM") as pp1,
                tc.tile_pool(name="ps2", bufs=1, space="PSUM") as pp2,
            ):
                for ns in range(NS):
                    hid = hp.tile([128, 32, 512], bf16, tag="hid", name=f"hid{ns}")
                    for mg in range(8):
                        w1t = wp.tile([128, 4, 8, 128], bf16, tag="w1", name=f"w1_{ns}_{mg}")
                        nc.sync.dma_start(w1t[:], w1b[:, mg * 4 : (mg + 1) * 4, :, :])
                        w2t = wp.tile([128, 4, 8, 128], bf16, tag="w2", name=f"w2_{ns}_{mg}")
                        nc.sync.dma_start(w2t[:], w2b[:, mg * 4 : (mg + 1) * 4, :, :])
                        for mj in range(4):
                            m = mg * 4 + mj
                            ph1 = pp1.tile([128, 512], f32, tag="ph1", name=f"ph1_{ns}_{m}")
                            ph2 = pp1.tile([128, 512], f32, tag="ph2", name=f"ph2_{ns}_{m}")
                            for dc in range(8):
                                nc.tensor.matmul(
                                    ph1[:], w1t[:, mj, dc, :], ei[:, dc, ts(ns, 512)],
                                    start=(dc == 0), stop=(dc == 7),
                                )
                            for dc in range(8):
                                nc.tensor.matmul(
                                    ph2[:], w2t[:, mj, dc, :], ei[:, dc, ts(ns, 512)],
                                    start=(dc == 0), stop=(dc == 7),
                                )
                            slt = sp.tile([128, 512], bf16, tag="sl", name=f"sl_{ns}_{m}")
                            nc.scalar.activation(slt[:], ph1[:], AF.Sigmoid)
                            tt = sp.tile([128, 512], bf16, tag="tt", name=f"tt_{ns}_{m}")
                            nc.vector.tensor_mul(tt[:], slt[:], ph1[:])
                            nc.vector.tensor_mul(hid[:, m, :], tt[:], ph2[:])

                    for sg in range(2):
                        eo = [
                            pp2.tile([128, D], f32, tag=f"eo{j}", name=f"eo_{ns}_{sg}_{j}")
                            for j in range(2)
                        ]
                        for hg in range(8):
                            w3t = wp3.tile(
                                [128, 4, D], bf16, tag="w3", name=f"w3_{ns}_{sg}_{hg}"
                            )
                            nc.sync.dma_start(w3t[:], w3b[hg, :, :, :])
                            for hj in range(4):
                                hc = hg * 4 + hj
                                for j in range(2):
                                    srel = sg * 2 + j
                                    for dsl in range(2):
                                        nc.tensor.matmul(
                                            eo[j][:, ts(dsl, 512)],
                                            hid[:, hc, ts(srel, 128)],
                                            w3t[:, hj, ts(dsl, 512)],
                                            start=(hc == 0), stop=(hc == 31),
                                        )
                        for j in range(2):
                            sglob = ns * 4 + sg * 2 + j
                            for k in range(2):
                                sc = scp.tile(
                                    [128, D], f32, tag="sc", name=f"sc_{sglob}_{k}"
                                )
                                nc.scalar.activation(
                                    sc[:], eo[j][:], AF.Copy,
                                    scale=gatesf[k][:, sglob : sglob + 1],
                                )
                                nidx = nc.gpsimd.value_load(
                                    ntile[k : k + 1, sglob : sglob + 1]
                                )
                                nc.gpsimd.dma_scatter_add(
                                    partial[:, :],
                                    sc[:].rearrange("p (o d) -> p o d", o=1),
                                    slw[k][:, sglob * 8 : (sglob + 1) * 8],
                                    num_idxs=128, num_idxs_reg=nidx, elem_size=D,
                                )

            # ================= reduce-scatter + output =================
            nc.gpsimd.collective_compute(
                "ReduceScatter", OP.add, RG, ins=[partial.opt()], outs=[rs_out.opt()]
            )
            nc.sync.dma_start(out, rs_out[:])

    nc.compile()
    return nc


def make_in_maps(x, norm_w, gate_w, w1, w2, w3):
    import ml_dtypes

    bf16 = ml_dtypes.bfloat16
    x = np.asarray(x, np.float32)
    norm_w = np.asarray(norm_w, np.float32)
    gate_w = np.asarray(gate_w, np.float32)
    w1 = np.asarray(w1, np.float32)
    w2 = np.asarray(w2, np.float32)
    w3 = np.asarray(w3, np.float32)

    xf = x.reshape(T, D)
    gweff = np.ascontiguousarray((gate_w * norm_w[None, :]).T)  # (D, E)
    ident = np.eye(128, dtype=np.float32)
    in_maps = []
    for c in range(8):
        w1e = (w1[c] * norm_w[:, None]).astype(bf16)
        w2e = (w2[c] * norm_w[:, None]).astype(bf16)
        w1s = np.ascontiguousarray(w1e.reshape(8, 128, 32, 128).transpose(1, 2, 0, 3))
        w2s = np.ascontiguousarray(w2e.reshape(8, 128, 32, 128).transpose(1, 2, 0, 3))
        w3s = np.ascontiguousarray(
            w3[c].astype(bf16).reshape(8, 4, 128, D).transpose(0, 2, 1, 3)
        )
        in_maps.append(
            {
                "xs": np.ascontiguousarray(xf[c * TPC : (c + 1) * TPC]),
                "gw": gweff,
                "w1b": w1s,
                "w2b": w2s,
                "w3b": w3s,
                "eid": np.full((16, 1), float(c), np.float32),
                "ident": ident,
            }
        )
    return in_maps


_NC = None


def _get_nc():
    global _NC
    if _NC is None:
        _NC = build_bass()
    return _NC


def run(x, norm_w, gate_w, w1, w2, w3, trace=False):
    from concourse.bass_utils import run_bass_kernel_spmd

    nc = _get_nc()
    in_maps = make_in_maps(x, norm_w, gate_w, w1, w2, w3)
    res = run_bass_kernel_spmd(nc, in_maps, core_ids=list(range(8)), trace=trace)
    outs = [res.results[c]["out"] for c in range(8)]
    full = np.concatenate(outs, axis=0).reshape(B, S, D).astype(np.float32)
    return full, res


def kernel(x, norm_w, gate_w, w1, w2, w3):
    full, _ = run(x, norm_w, gate_w, w1, w2, w3)
    return full

